# revision 19
# baseline (speedup 1.0000x reference)
"""Trainium2 Bass kernel for nn_Attention (GroupNorm + single-head 1x1-conv attention).

Sharding: 8 cores = 2 batches x 4 query-slices. Each core computes its batch's
full K/V (replicated within the 4-core batch group) and a 1024-column query
slice. GroupNorm is folded into the conv weights (hn = a*x + t), so the
normalize pass over x never happens: convs run directly on bf16 x with
per-batch rescaled weights.

Layouts (per core):  channels c = kt*128 + p
  K  [p, mt, j]   (c_out on partitions)  -> lhsT for S^T = K^T Q
  V^T [p(j), jb, o]                      -> lhsT for O = V A^T
  Q  [p, mt, i]                          -> rhs for S^T
  S^T [j, i] in PSUM -> exp on ACT -> A^T bf16; softmax denominator via
  ones-column matmul accumulated alongside; division applied to O columns at
  the end; V-bias folded in via a K=1 matmul against the denominator row.

walrus constraint: a Matmult may carry at most ONE semaphore wait. Contract
used here: (1) tiny "touch" matmuls pre-absorb every DMA-queue tick PE will
ever need (directly or via vector-clock inheritance); (2) all small PE scratch
outputs share ONE psum tensor so bank reuse stays same-tensor; (3) PSUM->SBUF
copies whose tiles feed matmuls are routed so operand + slot-release waits
land on one semaphore; (4) no SBUF pool is ever closed, so no instruction
inherits a pool-boundary clock.
"""

import numpy as np
import ml_dtypes

import concourse.bass as bass
import concourse.bacc as bacc
import concourse.tile as tile
from concourse import mybir
from concourse.bass_utils import run_bass_kernel_spmd

P = 128
CT = 4            # channel tiles (512 / 128)
C = 512
N = 4096          # spatial positions per batch
NB = 32           # j blocks (4096 / 128)
NCH = 8           # n chunks of 512 for K conv
IC = 2            # i chunks per core
ICW = 512         # i chunk width
F = 512
EPS = 1e-6
SCALE = float(C) ** -0.5

f32 = mybir.dt.float32
bf16 = mybir.dt.bfloat16

Alu = mybir.AluOpType
Act = mybir.ActivationFunctionType


def build_nc():
    nc = bacc.Bacc(None, target_bir_lowering=False)

    xb_p = nc.declare_dram_parameter("xb", [P, CT, N], bf16, isOutput=False)
    xqb_p = nc.declare_dram_parameter("xqb", [P, CT, IC * ICW], bf16, isOutput=False)
    xqr_p = nc.declare_dram_parameter("xqr", [P, CT, IC * ICW], f32, isOutput=False)
    wq_p = nc.declare_dram_parameter("wqT", [P, CT, F], bf16, isOutput=False)
    wk_p = nc.declare_dram_parameter("wkT", [P, CT, F], bf16, isOutput=False)
    wv_p = nc.declare_dram_parameter("wvT", [P, CT, F], bf16, isOutput=False)
    wp_p = nc.declare_dram_parameter("wpT", [P, CT, F], bf16, isOutput=False)
    gamma_p = nc.declare_dram_parameter("gamma2", [P, CT], f32, isOutput=False)
    beta_p = nc.declare_dram_parameter("beta2", [P, CT], f32, isOutput=False)
    qb_p = nc.declare_dram_parameter("qb2", [P, CT], f32, isOutput=False)
    kb_p = nc.declare_dram_parameter("kb2", [P, CT], f32, isOutput=False)
    pb_p = nc.declare_dram_parameter("pb2", [P, CT], f32, isOutput=False)
    vb_p = nc.declare_dram_parameter("vbrow", [1, F], f32, isOutput=False)
    selg_p = nc.declare_dram_parameter("selg", [P, CT, 32], f32, isOutput=False)
    selb_p = nc.declare_dram_parameter("selb", [32, F], f32, isOutput=False)
    out_p = nc.declare_dram_parameter("out", [P, CT, IC * ICW], f32, isOutput=True)

    with tile.TileContext(nc) as tc:
        with (
            tc.tile_pool(name="sb", bufs=1) as sb,
            tc.tile_pool(name="ps", bufs=1, space="PSUM") as ps,
        ):
            # ONE psum scratch tensor for all tiny matmul outputs.
            scratch = ps.tile([P, F], f32, tag="dtiny", name="scratch")

            def work(name):
                return ps.tile([P, F], f32, tag="work", bufs=3, name=name)

            # ---- persistent SBUF ----
            kmat = sb.tile([P, CT, N], bf16)
            vT = sb.tile([P, NB, F], bf16)
            qmat = sb.tile([P, CT, IC * ICW], bf16)
            wp_bf = sb.tile([P, CT, F], bf16)
            uq = sb.tile([P, CT], f32)
            uk = sb.tile([P, CT], f32)
            pb_t = sb.tile([P, CT], f32)
            uv_row = sb.tile([1, F], bf16)
            ones_col = sb.tile([P, 1], bf16)
            nc.vector.memset(ones_col, 1.0)
            ones_row = sb.tile([1, P], f32)
            nc.vector.memset(ones_row, 1.0)

            xb_t = sb.tile([P, CT, N], bf16)
            xqb_t = sb.tile([P, CT, IC * ICW], bf16)
            wraw = {}
            for name, p in (("q", wq_p), ("k", wk_p), ("v", wv_p), ("p", wp_p)):
                wraw[name] = sb.tile([P, CT, F], bf16, tag=f"wraw_{name}", name=f"wraw_{name}")
                nc.sync.dma_start(wraw[name], p[:])
            gamma_t = sb.tile([P, CT], f32, tag="gamma")
            beta_t = sb.tile([P, CT], f32, tag="beta")
            qb_t = sb.tile([P, CT], f32, tag="qb")
            kb_t = sb.tile([P, CT], f32, tag="kb")
            vb_row_t = sb.tile([1, F], f32, tag="vbrow")
            selg_t = sb.tile([P, CT, 32], f32, tag="selg")
            selb_t = sb.tile([32, F], f32, tag="selb")
            nc.sync.dma_start(gamma_t, gamma_p[:])
            nc.sync.dma_start(beta_t, beta_p[:])
            nc.sync.dma_start(qb_t, qb_p[:])
            nc.sync.dma_start(kb_t, kb_p[:])
            nc.sync.dma_start(vb_row_t, vb_p[:])
            nc.sync.dma_start(selg_t, selg_p[:])
            nc.sync.dma_start(selb_t, selb_p[:])
            nc.sync.dma_start(pb_t, pb_p[:])
            for kt in range(CT):
                nc.sync.dma_start(xb_t[:, kt, :], xb_p[:, kt, :])
            nc.sync.dma_start(xqb_t, xqb_p[:])

            # ---- touch matmuls: pre-absorb every DMA-queue wait on PE ----
            def touch(ap2d):
                m = min(128, ap2d.shape[1])
                nc.tensor.matmul(
                    scratch[0:m, 0:1], ap2d[:, 0:m], ap2d[:, 0:1],
                    start=True, stop=True,
                )

            touch(selg_t[:, 0, :])
            touch(selb_t)
            touch(wraw["q"][:, 0, :])
            touch(wraw["k"][:, 0, :])
            touch(wraw["v"][:, 0, :])
            touch(wraw["p"][:, 0, :])
            for kt in range(CT):
                touch(xb_t[:, kt, :])
            touch(xqb_t[:, 0, :])
            touch(gamma_t)
            touch(beta_t)
            touch(qb_t)
            touch(kb_t)
            touch(vb_row_t)

            # ---- Phase 1: per-channel stats (bn_stats), merge via PE ----
            stats6 = sb.tile([P, CT, NCH, 6], f32, tag="stats6")
            mv = sb.tile([P, CT, 2], f32, tag="mv")
            for kt in range(CT):
                for sub in range(NCH):
                    nc.vector.bn_stats(
                        stats6[:, kt, sub, :], xb_t[:, kt, sub * F:(sub + 1) * F]
                    )
                nc.vector.bn_aggr(mv[:, kt, :], stats6[:, kt])
            stats3 = sb.tile([P, CT, 3], f32, tag="stats3")
            for kt in range(CT):
                nc.vector.tensor_copy(stats3[:, kt, 0:2], mv[:, kt, :])
                nc.vector.tensor_mul(stats3[:, kt, 2:3], mv[:, kt, 0:1], mv[:, kt, 0:1])
            g3 = scratch[0:32, 0:3]
            for kt in range(CT):
                nc.tensor.matmul(
                    g3, selg_t[:, kt, :], stats3[:, kt, :],
                    start=(kt == 0), stop=(kt == CT - 1),
                )
            # group scalars: gtmp cols = [mean, var, mg^2, sd, r, r*mu]
            g3s = sb.tile([32, 3], f32, tag="g3s")
            nc.vector.tensor_copy(g3s, g3)
            gtmp = sb.tile([32, 6], f32, tag="gtmp")
            nc.vector.tensor_copy(gtmp[:, 0:1], g3s[:, 0:1])
            nc.vector.tensor_add(gtmp[:, 1:2], g3s[:, 1:2], g3s[:, 2:3])
            nc.vector.tensor_mul(gtmp[:, 2:3], g3s[:, 0:1], g3s[:, 0:1])
            nc.vector.tensor_sub(gtmp[:, 1:2], gtmp[:, 1:2], gtmp[:, 2:3])
            eps_t = sb.tile([32, 1], f32, tag="eps")
            nc.vector.memset(eps_t, EPS)
            nc.scalar.activation(gtmp[:, 3:4], gtmp[:, 1:2], Act.Sqrt, bias=eps_t, scale=1.0)
            nc.vector.reciprocal(gtmp[:, 4:5], gtmp[:, 3:4])
            nc.vector.tensor_mul(gtmp[:, 5:6], gtmp[:, 4:5], gtmp[:, 0:1])
            # broadcast (r, r*mu) from groups to channels via PE
            rc = sb.tile([P, CT, 2], f32, tag="rc")
            for mt in range(CT):
                cb = scratch[0:P, 8 + 2 * mt:10 + 2 * mt]
                nc.tensor.matmul(
                    cb, selb_t[:, mt * P:(mt + 1) * P], gtmp[:, 4:6],
                    start=True, stop=True,
                )
                nc.vector.tensor_copy(rc[:, mt, :], cb)
            a_t = sb.tile([P, CT], f32, tag="a")
            t_t = sb.tile([P, CT], f32, tag="t")
            aq_t = sb.tile([P, CT], f32, tag="aq")
            t_bf = sb.tile([P, CT], bf16, tag="t_bf")
            nc.vector.tensor_mul(a_t, gamma_t, rc[:, :, 0])
            nc.vector.tensor_mul(t_t, gamma_t, rc[:, :, 1])
            nc.vector.tensor_sub(t_t, beta_t, t_t)
            nc.vector.tensor_scalar_mul(aq_t, a_t, SCALE)
            nc.vector.tensor_copy(t_bf, t_t)

            # ---- Phase 2: folded weights (order q, v, then k LAST so the
            # first conv matmul's single DVE wait covers all of them) ----
            wq_bf = sb.tile([P, CT, F], bf16, tag="wq_bf")
            wk_bf = sb.tile([P, CT, F], bf16, tag="wk_bf")
            wv_bf = sb.tile([P, CT, F], bf16, tag="wv_bf")
            nc.vector.tensor_copy(wp_bf, wraw["p"])
            for kt in range(CT):
                nc.vector.tensor_scalar_mul(wq_bf[:, kt, :], wraw["q"][:, kt, :], aq_t[:, kt:kt + 1])
            for kt in range(CT):
                nc.vector.tensor_scalar_mul(wv_bf[:, kt, :], wraw["v"][:, kt, :], a_t[:, kt:kt + 1])
            for kt in range(CT):
                nc.vector.tensor_scalar_mul(wk_bf[:, kt, :], wraw["k"][:, kt, :], a_t[:, kt:kt + 1])
            # u_q = (Wq@t + bq)*scale ; u_k = Wk@t + bk ; u_v as a row
            for mt in range(CT):
                up = scratch[0:P, 16 + mt:17 + mt]
                for kt in range(CT):
                    nc.tensor.matmul(
                        up, wraw["q"][:, kt, mt * P:(mt + 1) * P], t_bf[:, kt:kt + 1],
                        start=(kt == 0), stop=(kt == CT - 1),
                    )
                nc.vector.tensor_scalar(
                    uq[:, mt:mt + 1], up, qb_t[:, mt:mt + 1], SCALE,
                    op0=Alu.add, op1=Alu.mult,
                )
                up2 = scratch[0:P, 20 + mt:21 + mt]
                for kt in range(CT):
                    nc.tensor.matmul(
                        up2, wraw["k"][:, kt, mt * P:(mt + 1) * P], t_bf[:, kt:kt + 1],
                        start=(kt == 0), stop=(kt == CT - 1),
                    )
                nc.vector.tensor_add(uk[:, mt:mt + 1], up2, kb_t[:, mt:mt + 1])
            uvp = scratch[0:1, 0:F]
            for kt in range(CT):
                nc.tensor.matmul(
                    uvp, t_bf[:, kt:kt + 1], wraw["v"][:, kt, :],
                    start=(kt == 0), stop=(kt == CT - 1),
                )
            nc.vector.tensor_add(uv_row, uvp, vb_row_t)

            # ---- Phase 3: convs (PSUM -> SBUF copies all on ACT) ----
            for mt in range(CT):  # K = Wk~ @ x   [c_out, j]
                for ch in range(NCH):
                    pk = work(f"pk{mt}_{ch}")
                    for kt in range(CT):
                        nc.tensor.matmul(
                            pk,
                            wk_bf[:, kt, mt * P:(mt + 1) * P],
                            xb_t[:, kt, ch * F:(ch + 1) * F],
                            start=(kt == 0), stop=(kt == CT - 1),
                        )
                    nc.scalar.activation(
                        kmat[:, mt, ch * F:(ch + 1) * F], pk, Act.Identity,
                        bias=uk[:, mt:mt + 1], scale=1.0,
                    )
            for jb in range(NB):  # V^T = x^T @ Wv~^T   [j, c_out]
                pv = work(f"pv{jb}")
                for kt in range(CT):
                    nc.tensor.matmul(
                        pv,
                        xb_t[:, kt, jb * P:(jb + 1) * P],
                        wv_bf[:, kt, :],
                        start=(kt == 0), stop=(kt == CT - 1),
                    )
                nc.scalar.copy(vT[:, jb, :], pv)
            for mt in range(CT):  # Q = Wq~ @ xq   [c_out, i]
                for ch2 in range(IC):
                    pq = work(f"pq{mt}_{ch2}")
                    for kt in range(CT):
                        nc.tensor.matmul(
                            pq,
                            wq_bf[:, kt, mt * P:(mt + 1) * P],
                            xqb_t[:, kt, ch2 * ICW:(ch2 + 1) * ICW],
                            start=(kt == 0), stop=(kt == CT - 1),
                        )
                    nc.scalar.activation(
                        qmat[:, mt, ch2 * ICW:(ch2 + 1) * ICW], pq, Act.Identity,
                        bias=uq[:, mt:mt + 1], scale=1.0,
                    )

            # absorb the last phase-2 DVE tick (uv_row) so later matmuls that
            # inherit it via vector clocks never need an extra DVE wait
            touch(uv_row)

            # ---- Phase 4: attention + proj, per i-chunk ----
            O_ps = [
                ps.tile([P, ICW], f32, tag=f"O{mt}", name=f"O{mt}")
                for mt in range(CT)
            ]
            d_ps = scratch[0:1, 0:ICW]
            for ic in range(IC):
                for jb in range(NB):
                    st = work(f"st{ic}_{jb}")
                    for mt4 in range(CT):
                        nc.tensor.matmul(
                            st,
                            kmat[:, mt4, jb * P:(jb + 1) * P],
                            qmat[:, mt4, ic * ICW:(ic + 1) * ICW],
                            start=(mt4 == 0), stop=(mt4 == CT - 1),
                        )
                    ex = sb.tile([P, ICW], bf16, tag="ex", bufs=3, name=f"ex{ic}_{jb}")
                    nc.scalar.activation(ex, st, Act.Exp)
                    for mt in range(CT):
                        nc.tensor.matmul(
                            O_ps[mt], vT[:, jb, mt * P:(mt + 1) * P], ex,
                            start=(jb == 0), stop=False,
                        )
                    nc.tensor.matmul(
                        d_ps, ones_col, ex, start=(jb == 0), stop=(jb == NB - 1)
                    )
                # denominator to SBUF: both copies on ACT so the psum's reader
                # set (and downstream matmul waits) stay on one proc
                d_sb = sb.tile([1, ICW], f32, tag="dsb", bufs=2, name=f"dsb{ic}")
                nc.scalar.copy(d_sb, d_ps)
                d_bf = sb.tile([1, ICW], bf16, tag="dbf", bufs=2, name=f"dbf{ic}")
                nc.scalar.copy(d_bf, d_ps)
                for mt in range(CT):
                    nc.tensor.matmul(
                        O_ps[mt], uv_row[:, mt * P:(mt + 1) * P], d_bf,
                        start=False, stop=True,
                    )
                # O -> SBUF via ACT (slot release merges with exp waits)
                O_sb = sb.tile([P, CT, ICW], f32, tag="O_sb", name=f"Osb{ic}")
                for mt in range(CT):
                    nc.scalar.copy(O_sb[:, mt, :], O_ps[mt])
                # 1/denom, broadcast down partitions via a K=1 PE matmul
                recip = sb.tile([1, ICW], f32, tag="recip", bufs=2, name=f"recip{ic}")
                nc.vector.reciprocal(recip, d_sb)
                rb_ps = work(f"rb{ic}")
                nc.tensor.matmul(rb_ps, ones_row, recip, start=True, stop=True)
                # absorb the fresh ACT ticks (O_sb copies) on PE so the proj
                # matmuls (reading osb) only ever need their single DVE wait
                touch(O_sb[:, CT - 1, :])
                osb = sb.tile([P, CT, ICW], bf16, tag="osb", name=f"osbb{ic}")
                for mt in range(CT):
                    nc.vector.tensor_mul(osb[:, mt, :], O_sb[:, mt, :], rb_ps)
                # proj + bias + residual
                xqr_t = sb.tile([P, CT, ICW], f32, tag="xqr", name=f"xqr{ic}")
                nc.sync.dma_start(xqr_t, xqr_p[:, :, ic * ICW:(ic + 1) * ICW])
                outt = sb.tile([P, CT, ICW], f32, tag="outt", name=f"outt{ic}")
                for mt in range(CT):
                    pp = work(f"pp{ic}_{mt}")
                    for kt in range(CT):
                        nc.tensor.matmul(
                            pp, wp_bf[:, kt, mt * P:(mt + 1) * P], osb[:, kt, :],
                            start=(kt == 0), stop=(kt == CT - 1),
                        )
                    nc.vector.scalar_tensor_tensor(
                        outt[:, mt, :], pp, pb_t[:, mt:mt + 1],
                        xqr_t[:, mt, :],
                        op0=Alu.add, op1=Alu.add,
                    )
                nc.sync.dma_start(out_p[:, :, ic * ICW:(ic + 1) * ICW], outt)

    nc.finalize()

    # walrus constraint: a Matmult may carry at most one semaphore wait
    bad = []
    for name, inst in nc.inst_map.items():
        if isinstance(inst, mybir.InstMatmult):
            si = inst.sync_info
            nw = len(si.on_wait) if si and si.on_wait else 0
            if nw > 1:
                bad.append((name, [w.ant_name for w in si.on_wait]))
    if bad:
        raise RuntimeError(f"matmuls with >1 wait: {bad}")
    return nc


_NC = None


def _get_nc():
    global _NC
    if _NC is None:
        _NC = build_nc()
    return _NC


def _stripe(a):
    # [512, m...] -> [128, 4, m...] with c = kt*128 + p
    return np.ascontiguousarray(a.reshape(CT, P, *a.shape[1:]).transpose(1, 0, *range(2, a.ndim + 1)))


def _stripe_vec(v):
    # [512] -> [128, 4]
    return np.ascontiguousarray(v.reshape(CT, P).T)


def make_in_maps(x, gn_gamma, gn_beta, q_w, q_b, k_w, k_b, v_w, v_b, proj_w, proj_b):
    bfl = ml_dtypes.bfloat16
    x = np.asarray(x, dtype=np.float32)
    selg = np.zeros((C, 32), np.float32)
    selg[np.arange(C), np.arange(C) // 16] = 1.0 / 16.0
    selb = np.zeros((32, C), np.float32)
    selb[np.arange(C) // 16, np.arange(C)] = 1.0

    common = {
        "wqT": _stripe(np.ascontiguousarray(np.asarray(q_w, np.float32).T)).astype(bfl),
        "wkT": _stripe(np.ascontiguousarray(np.asarray(k_w, np.float32).T)).astype(bfl),
        "wvT": _stripe(np.ascontiguousarray(np.asarray(v_w, np.float32).T)).astype(bfl),
        "wpT": _stripe(np.ascontiguousarray(np.asarray(proj_w, np.float32).T)).astype(bfl),
        "gamma2": _stripe_vec(np.asarray(gn_gamma, np.float32)),
        "beta2": _stripe_vec(np.asarray(gn_beta, np.float32)),
        "qb2": _stripe_vec(np.asarray(q_b, np.float32)),
        "kb2": _stripe_vec(np.asarray(k_b, np.float32)),
        "pb2": _stripe_vec(np.asarray(proj_b, np.float32)),
        "vbrow": np.asarray(v_b, np.float32).reshape(1, C).copy(),
        "selg": _stripe(selg),
        "selb": selb,
    }
    in_maps = []
    for core in range(8):
        b = core // 4
        s = (core % 4) * (IC * ICW)
        xb = x[b].reshape(C, N)
        im = dict(common)
        im["xb"] = _stripe(xb).astype(bfl)
        im["xqb"] = _stripe(np.ascontiguousarray(xb[:, s:s + IC * ICW])).astype(bfl)
        im["xqr"] = _stripe(np.ascontiguousarray(xb[:, s:s + IC * ICW]))
        in_maps.append(im)
    return in_maps


def assemble_output(results, x_shape=(2, C, 64, 64)):
    full = np.empty((2, C, N), np.float32)
    for core in range(8):
        b = core // 4
        s = (core % 4) * (IC * ICW)
        o = results[core]["out"]  # [128, 4, 1024]
        full[b][:, s:s + IC * ICW] = np.asarray(o).transpose(1, 0, 2).reshape(C, IC * ICW)
    return full.reshape(*x_shape)


def run(trace=False, **inputs):
    nc = _get_nc()
    in_maps = make_in_maps(**inputs)
    res = run_bass_kernel_spmd(nc, in_maps, list(range(8)), trace=trace)
    out = assemble_output(res.results, tuple(np.asarray(inputs["x"]).shape))
    return out, res


def kernel(**inputs):
    out, _ = run(trace=False, **inputs)
    return out


# revision 20
# speedup vs baseline: 1.2839x; 1.2839x over previous
"""Trainium2 Bass kernel for nn_Attention (GroupNorm + single-head 1x1-conv attention).

Sharding: 8 cores = 2 batches x 4 query-slices. Each core computes its batch's
full K/V (replicated within the 4-core batch group) and a 1024-column query
slice.

GroupNorm is folded into the conv weights ON THE HOST (hn = a*x + t, so
W@hn + b == (W*a)@x + (W@t + b)): the device runs convs directly on bf16 x
with per-batch folded weights and biases. The device kernel is:
  convs (K, V^T, Q) -> attention (S^T = K^T Q per j-block, exp on ACT,
  O += V^T A^T accumulated in PSUM, softmax denominator accumulated on DVE)
  -> divide O columns by denominator -> proj + bias + residual -> out.

Layouts (per core):  channels c = kt*128 + p
  K  [p, mt, j]   (c_out on partitions)  -> lhsT for S^T = K^T Q
  V^T [p(j), jb, o]                      -> lhsT for O = V A^T
  Q  [p, mt, i]                          -> rhs for S^T
  V-bias folded in via a K=1 matmul against the denominator row.
"""

import numpy as np
import ml_dtypes

import concourse.bacc as bacc
import concourse.tile as tile
from concourse import mybir
from concourse.bass_utils import run_bass_kernel_spmd

P = 128
CT = 4            # channel tiles (512 / 128)
C = 512
N = 4096          # spatial positions per batch
NB = 32           # j blocks (4096 / 128)
NCH = 8           # n chunks of 512 for K conv
IC = 2            # i chunks per core
ICW = 512         # i chunk width
F = 512
EPS = 1e-6
SCALE = float(C) ** -0.5

f32 = mybir.dt.float32
bf16 = mybir.dt.bfloat16

Alu = mybir.AluOpType
Act = mybir.ActivationFunctionType


def build_nc():
    nc = bacc.Bacc(None, target_bir_lowering=False)

    xb_p = nc.declare_dram_parameter("xb", [P, CT, N], bf16, isOutput=False)
    xqb_p = nc.declare_dram_parameter("xqb", [P, CT, IC * ICW], bf16, isOutput=False)
    xqr_p = nc.declare_dram_parameter("xqr", [P, CT, IC * ICW], f32, isOutput=False)
    wq_p = nc.declare_dram_parameter("wqT", [P, CT, F], bf16, isOutput=False)
    wk_p = nc.declare_dram_parameter("wkT", [P, CT, F], bf16, isOutput=False)
    wv_p = nc.declare_dram_parameter("wvT", [P, CT, F], bf16, isOutput=False)
    wp_p = nc.declare_dram_parameter("wpT", [P, CT, F], bf16, isOutput=False)
    uq_p = nc.declare_dram_parameter("uq2", [P, CT], f32, isOutput=False)
    uk_p = nc.declare_dram_parameter("uk2", [P, CT], f32, isOutput=False)
    pb_p = nc.declare_dram_parameter("pb2", [P, CT], f32, isOutput=False)
    uv_p = nc.declare_dram_parameter("uvrow", [1, F], bf16, isOutput=False)
    out_p = nc.declare_dram_parameter("out", [P, CT, IC * ICW], f32, isOutput=True)

    with tile.TileContext(nc) as tc:
        with (
            tc.tile_pool(name="sb", bufs=1) as sb,
            tc.tile_pool(name="ps", bufs=1, space="PSUM") as ps,
        ):
            def work(name):
                return ps.tile([P, F], f32, tag="work", bufs=4, name=name)

            # ---- persistent SBUF ----
            kmat = sb.tile([P, CT, N], bf16)
            vT = sb.tile([P, NB, F], bf16)
            qmat = sb.tile([P, CT, IC * ICW], bf16)
            uq = sb.tile([P, CT], f32)
            uk = sb.tile([P, CT], f32)
            pb_t = sb.tile([P, CT], f32)
            uv_row = sb.tile([1, F], bf16)
            ones_col = sb.tile([P, 1], f32)
            nc.vector.memset(ones_col, 1.0)
            ones_row = sb.tile([1, P], f32)
            nc.vector.memset(ones_row, 1.0)

            wq_t = sb.tile([P, CT, F], bf16)
            wk_t = sb.tile([P, CT, F], bf16)
            wv_t = sb.tile([P, CT, F], bf16)
            wp_t = sb.tile([P, CT, F], bf16)
            xb_t = sb.tile([P, CT, N], bf16)
            xqb_t = sb.tile([P, CT, IC * ICW], bf16)

            nc.sync.dma_start(wk_t, wk_p[:])
            nc.sync.dma_start(wv_t, wv_p[:])
            nc.sync.dma_start(wq_t, wq_p[:])
            nc.sync.dma_start(wp_t, wp_p[:])
            nc.sync.dma_start(uk, uk_p[:])
            nc.sync.dma_start(uq, uq_p[:])
            nc.sync.dma_start(pb_t, pb_p[:])
            nc.sync.dma_start(uv_row, uv_p[:])
            # x chunks ordered so early conv chunks land first
            for chh in range(4):
                for kt in range(CT):
                    nc.sync.dma_start(
                        xb_t[:, kt, chh * 1024:(chh + 1) * 1024],
                        xb_p[:, kt, chh * 1024:(chh + 1) * 1024],
                    )
            nc.sync.dma_start(xqb_t, xqb_p[:])

            # ---- Phase A: convs (PSUM -> SBUF copies on ACT, with bias) ----
            for ch in range(NCH):      # K = Wk~ @ x   [c_out, j]
                for mt in range(CT):
                    pk = work(f"pk{ch}_{mt}")
                    for kt in range(CT):
                        nc.tensor.matmul(
                            pk,
                            wk_t[:, kt, mt * P:(mt + 1) * P],
                            xb_t[:, kt, ch * F:(ch + 1) * F],
                            start=(kt == 0), stop=(kt == CT - 1),
                        )
                    nc.scalar.activation(
                        kmat[:, mt, ch * F:(ch + 1) * F], pk, Act.Identity,
                        bias=uk[:, mt:mt + 1], scale=1.0,
                    )
            for jb in range(NB):       # V^T = x^T @ Wv~^T   [j, c_out]
                pv = work(f"pv{jb}")
                for kt in range(CT):
                    nc.tensor.matmul(
                        pv,
                        xb_t[:, kt, jb * P:(jb + 1) * P],
                        wv_t[:, kt, :],
                        start=(kt == 0), stop=(kt == CT - 1),
                    )
                nc.scalar.copy(vT[:, jb, :], pv)
            for mt in range(CT):       # Q = Wq~ @ xq   [c_out, i]
                for ch2 in range(IC):
                    pq = work(f"pq{mt}_{ch2}")
                    for kt in range(CT):
                        nc.tensor.matmul(
                            pq,
                            wq_t[:, kt, mt * P:(mt + 1) * P],
                            xqb_t[:, kt, ch2 * ICW:(ch2 + 1) * ICW],
                            start=(kt == 0), stop=(kt == CT - 1),
                        )
                    nc.scalar.activation(
                        qmat[:, mt, ch2 * ICW:(ch2 + 1) * ICW], pq, Act.Identity,
                        bias=uq[:, mt:mt + 1], scale=1.0,
                    )

            # ---- Phase B: attention + proj, per i-chunk ----
            O_ps = [
                ps.tile([P, ICW], f32, tag=f"O{mt}", name=f"O{mt}")
                for mt in range(CT)
            ]
            for ic in range(IC):
                acc_d = sb.tile([P, ICW], f32, tag="acc_d", bufs=2, name=f"acc{ic}")
                nc.vector.memset(acc_d, 0.0)
                for jb in range(NB):
                    st = work(f"st{ic}_{jb}")
                    for mt4 in range(CT):
                        nc.tensor.matmul(
                            st,
                            kmat[:, mt4, jb * P:(jb + 1) * P],
                            qmat[:, mt4, ic * ICW:(ic + 1) * ICW],
                            start=(mt4 == 0), stop=(mt4 == CT - 1),
                        )
                    ex = sb.tile([P, ICW], bf16, tag="ex", bufs=3, name=f"ex{ic}_{jb}")
                    nc.scalar.activation(ex, st, Act.Exp)
                    for mt in range(CT):
                        nc.tensor.matmul(
                            O_ps[mt], vT[:, jb, mt * P:(mt + 1) * P], ex,
                            start=(jb == 0), stop=False,
                        )
                    nc.vector.tensor_add(acc_d, acc_d, ex)
                # denominator: cross-partition reduce via fp32 matmul
                dred = work(f"dred{ic}")
                nc.tensor.matmul(dred[0:1, :], ones_col, acc_d, start=True, stop=True)
                d_sb = sb.tile([1, ICW], f32, tag="dsb", bufs=2, name=f"dsb{ic}")
                nc.scalar.copy(d_sb, dred[0:1, :])
                d_bf = sb.tile([1, ICW], bf16, tag="dbf", bufs=2, name=f"dbf{ic}")
                nc.scalar.copy(d_bf, dred[0:1, :])
                # V-bias: O += u_v^T @ denom  (closes the accumulation groups)
                for mt in range(CT):
                    nc.tensor.matmul(
                        O_ps[mt], uv_row[:, mt * P:(mt + 1) * P], d_bf,
                        start=False, stop=True,
                    )
                # broadcast denom down partitions (fp32 K=1 matmul), then
                # reciprocal on the wide tile (cheap: 128 lanes)
                rb_ps = work(f"rb{ic}")
                nc.tensor.matmul(rb_ps, ones_row, d_sb, start=True, stop=True)
                rb_sb = sb.tile([P, ICW], f32, tag="rb", bufs=2, name=f"rbs{ic}")
                nc.vector.reciprocal(rb_sb, rb_ps)
                # O -> SBUF via ACT, then scale columns by 1/denom -> bf16
                O_sb = sb.tile([P, CT, ICW], f32, tag="O_sb", name=f"Osb{ic}")
                for mt in range(CT):
                    nc.scalar.copy(O_sb[:, mt, :], O_ps[mt])
                osb = sb.tile([P, CT, ICW], bf16, tag="osb", name=f"osbb{ic}")
                for mt in range(CT):
                    nc.vector.tensor_mul(osb[:, mt, :], O_sb[:, mt, :], rb_sb)
                # proj + bias + residual
                xqr_t = sb.tile([P, CT, ICW], f32, tag="xqr", name=f"xqr{ic}")
                nc.sync.dma_start(xqr_t, xqr_p[:, :, ic * ICW:(ic + 1) * ICW])
                outt = sb.tile([P, CT, ICW], f32, tag="outt", name=f"outt{ic}")
                for mt in range(CT):
                    pp = work(f"pp{ic}_{mt}")
                    for kt in range(CT):
                        nc.tensor.matmul(
                            pp, wp_t[:, kt, mt * P:(mt + 1) * P], osb[:, kt, :],
                            start=(kt == 0), stop=(kt == CT - 1),
                        )
                    nc.vector.scalar_tensor_tensor(
                        outt[:, mt, :], pp, pb_t[:, mt:mt + 1],
                        xqr_t[:, mt, :],
                        op0=Alu.add, op1=Alu.add,
                    )
                nc.sync.dma_start(out_p[:, :, ic * ICW:(ic + 1) * ICW], outt)

    nc.finalize()
    return nc


_NC = None


def _get_nc():
    global _NC
    if _NC is None:
        _NC = build_nc()
    return _NC


def _stripe(a):
    # [512, m...] -> [128, 4, m...] with c = kt*128 + p
    return np.ascontiguousarray(a.reshape(CT, P, *a.shape[1:]).transpose(1, 0, *range(2, a.ndim + 1)))


def _stripe_vec(v):
    # [512] -> [128, 4]
    return np.ascontiguousarray(v.reshape(CT, P).T)


def make_in_maps(x, gn_gamma, gn_beta, q_w, q_b, k_w, k_b, v_w, v_b, proj_w, proj_b):
    bfl = ml_dtypes.bfloat16
    x = np.asarray(x, dtype=np.float32)
    gn_gamma = np.asarray(gn_gamma, np.float32)
    gn_beta = np.asarray(gn_beta, np.float32)
    q_w = np.asarray(q_w, np.float32)
    k_w = np.asarray(k_w, np.float32)
    v_w = np.asarray(v_w, np.float32)
    proj_w = np.asarray(proj_w, np.float32)

    in_maps = [None] * 8
    for b in range(2):
        xb = x[b].reshape(C, N)
        xbb = xb.astype(bfl)           # device sees bf16 x; fold stats match
        xbf = xbb.astype(np.float32)
        # GroupNorm stats per group of 16 channels
        xg = xbf.reshape(32, 16 * N)
        mg = xg.mean(axis=1)
        vg = xg.var(axis=1)
        r = 1.0 / np.sqrt(vg + EPS)
        a = gn_gamma * np.repeat(r, 16)
        t = gn_beta - gn_gamma * np.repeat(r * mg, 16)
        # fold into weights/biases
        wq = q_w * (a * SCALE)[None, :]
        uqv = ((q_w @ t) + np.asarray(q_b, np.float32)) * SCALE
        wk = k_w * a[None, :]
        ukv = (k_w @ t) + np.asarray(k_b, np.float32)
        wv = v_w * a[None, :]
        uvv = (v_w @ t) + np.asarray(v_b, np.float32)

        common = {
            "wqT": _stripe(np.ascontiguousarray(wq.T)).astype(bfl),
            "wkT": _stripe(np.ascontiguousarray(wk.T)).astype(bfl),
            "wvT": _stripe(np.ascontiguousarray(wv.T)).astype(bfl),
            "wpT": _stripe(np.ascontiguousarray(proj_w.T)).astype(bfl),
            "uq2": _stripe_vec(uqv.astype(np.float32)),
            "uk2": _stripe_vec(ukv.astype(np.float32)),
            "pb2": _stripe_vec(np.asarray(proj_b, np.float32)),
            "uvrow": uvv.reshape(1, C).astype(bfl),
            "xb": _stripe(xbb),
        }
        for q in range(4):
            s = q * (IC * ICW)
            im = dict(common)
            im["xqb"] = np.ascontiguousarray(common["xb"][:, :, s:s + IC * ICW])
            im["xqr"] = _stripe(np.ascontiguousarray(xb[:, s:s + IC * ICW]))
            in_maps[b * 4 + q] = im
    return in_maps


def assemble_output(results, x_shape=(2, C, 64, 64)):
    full = np.empty((2, C, N), np.float32)
    for core in range(8):
        b = core // 4
        s = (core % 4) * (IC * ICW)
        o = results[core]["out"]  # [128, 4, 1024]
        full[b][:, s:s + IC * ICW] = np.asarray(o).transpose(1, 0, 2).reshape(C, IC * ICW)
    return full.reshape(*x_shape)


def run(trace=False, **inputs):
    nc = _get_nc()
    in_maps = make_in_maps(**inputs)
    res = run_bass_kernel_spmd(nc, in_maps, list(range(8)), trace=trace)
    out = assemble_output(res.results, tuple(np.asarray(inputs["x"]).shape))
    return out, res


def kernel(**inputs):
    out, _ = run(trace=False, **inputs)
    return out
